# revision 24
# baseline (speedup 1.0000x reference)
# Differential GQA attention layer (B=2, S=1024, E=2048, H=16, KVH=4, D=128)
# distributed over 8 TRN2 NeuronCores: shard = (batch b, kv-group g) so each
# core owns 1 batch x 4 query heads (1 kv head). All attention is core-local;
# the Wo row-sharded output projection partials are summed on the host.
#
# Self-contained: hardcodes shapes/sharding; builds+compiles a Bass/Tile
# kernel on first call and runs it via run_bass_kernel_spmd on cores 0-7.
import numpy as np

B, S, E, H, KVH = 2, 1024, 2048, 16, 4
D = 128
NEG = -1e30
LAM_INIT = 0.2  # 0.8 - 0.6*exp(-0.3*layer_idx), layer_idx=0
NCORES = 8
HPC = H // KVH  # heads per core = 4

MM_DT = "bf16"  # PE path dtype: bf16 LDWEIGHTS+MM is ~1.9x faster than f32/f32r
NBLK = 2 + 1 + 2 * HPC  # weight column blocks: k0,k1,v,q0..q7

_cache = {}


def _build(dbg=False):
    import concourse.mybir as mybir
    import concourse.tile as tile
    from concourse import bacc
    from concourse.masks import make_identity
    from contextlib import ExitStack

    F32 = mybir.dt.float32
    BF16 = mybir.dt.bfloat16
    MMD = BF16 if MM_DT == "bf16" else (mybir.dt.float32r if MM_DT == "f32r" else F32)
    ALU = mybir.AluOpType
    ACT = mybir.ActivationFunctionType

    nc = bacc.Bacc(None, target_bir_lowering=False)

    xT = nc.declare_dram_parameter("xT", [E, S], MMD, isOutput=False)
    # all projection weights, e-interleaved per 128-col block:
    # Wil[p, blk*2048 + e*128 + c] = W[e*128+p, blk_cols+c]
    Wil = nc.declare_dram_parameter("Wil", [128, NBLK * 2048], MMD, isOutput=False)
    Wo = nc.declare_dram_parameter("Wo", [HPC * D, E], MMD, isOutput=False)
    cosd = nc.declare_dram_parameter("cosd", [2 * D, S], BF16, isOutput=False)
    sind = nc.declare_dram_parameter("sind", [2 * D, S], BF16, isOutput=False)
    lamn = nc.declare_dram_parameter("lamn", [D, HPC], F32, isOutput=False)
    maskn = nc.declare_dram_parameter("maskn", [D, D], F32, isOutput=False)
    out_ext = nc.declare_dram_parameter("out", [S, E], BF16, isOutput=True)

    ISCALE = 1.0 / float(np.sqrt(D))
    NQT = S // 128
    NPC = S // 512

    with tile.TileContext(nc) as tc:
        with ExitStack() as ctx:
            cpool = ctx.enter_context(tc.tile_pool(name="const", bufs=1))
            qkpool = ctx.enter_context(tc.tile_pool(name="qk", bufs=1))
            smalls = ctx.enter_context(tc.tile_pool(name="smalls", bufs=2))

            # constants (tiles now; DMAs deferred until after the first
            # m1 block so xT/W loads own the queues at kernel start)
            cos_t = [cpool.tile([128, S], F32, tag=f"cos{a}", name=f"cos{a}") for a in range(2)]
            sin_t = [cpool.tile([128, S], F32, tag=f"sin{a}", name=f"sin{a}") for a in range(2)]
            cos_b = [cpool.tile([128, S], BF16, tag=f"cosb{a}", name=f"cosb{a}") for a in range(2)]
            sin_b = [cpool.tile([128, S], BF16, tag=f"sinb{a}", name=f"sinb{a}") for a in range(2)]
            lam_t = cpool.tile([128, HPC], F32, tag="lam", name="lam")
            mask_t = cpool.tile([128, 128], F32, tag="mask", name="mask")

            def load_consts():
                for a in range(2):
                    nc.sync.dma_start(out=cos_b[a][:], in_=cosd[a * 128:(a + 1) * 128, :])
                    nc.sync.dma_start(out=sin_b[a][:], in_=sind[a * 128:(a + 1) * 128, :])
                nc.sync.dma_start(out=lam_t[:], in_=lamn[:])
                nc.sync.dma_start(out=mask_t[:], in_=maskn[:])
                for a in range(2):
                    nc.scalar.copy(cos_t[a][:], cos_b[a][:])
                    nc.scalar.copy(sin_t[a][:], sin_b[a][:])
                nc.scalar.copy(mask_b[:], mask_t[:])
            ident = cpool.tile([128, 128], F32, tag="ident", name="ident")
            make_identity(nc, ident[:])
            identb = cpool.tile([128, 128], BF16, tag="identb", name="identb")
            make_identity(nc, identb[:])
            ones_t = cpool.tile([128, 128], BF16, tag="ones", name="ones")
            nc.gpsimd.memset(ones_t[:], 1.0)
            mask_b = cpool.tile([128, 128], BF16, tag="maskb", name="maskb")

            # persistent activations
            qT = [[qkpool.tile([128, S], MMD, tag=f"qT{h}{a}", name=f"qT{h}{a}")
                   for a in range(2)] for h in range(HPC)]
            kT = [qkpool.tile([128, S], MMD, tag=f"kT{a}", name=f"kT{a}") for a in range(2)]
            vT = qkpool.tile([128, S], MMD, tag="vT", name="vT")
            v_t = [qkpool.tile([128, 128], BF16, tag=f"v{j}", name=f"v{j}")
                   for j in range(NQT)]
            attf = [[qkpool.tile([128, 512], MMD, tag=f"attf{h}{c}", name=f"attf{h}{c}")
                     for c in range(NPC)] for h in range(HPC)]
            wo_t = [qkpool.tile([128, E], MMD, tag=f"wo{h}", name=f"wo{h}")
                    for h in range(HPC)]

            # ------------- phase A: x @ W -> qT/kT/vT (+rope), v -------------
            with ExitStack() as actx:
                xpool = actx.enter_context(tc.tile_pool(name="xT", bufs=16))
                wpool = actx.enter_context(tc.tile_pool(name="w", bufs=3))
                m1ps = actx.enter_context(tc.tile_pool(name="m1ps", bufs=6, space="PSUM"))
                vtrps = actx.enter_context(tc.tile_pool(name="vtrps", bufs=2, space="PSUM"))
                rtmp = actx.enter_context(tc.tile_pool(name="rtmp", bufs=3))

                # k and v first so phase B can start while q streams
                blocks = []
                for a in range(2):
                    blocks.append(("k", a))
                blocks.append(("v", None))
                for h in range(HPC):
                    for a in range(2):
                        blocks.append(("q", (h, a)))

                # xT tiles on the gpsimd DMA queue (parallel with weights
                # on the sync queue)
                xt = []
                for e in range(16):
                    t = xpool.tile([128, S], MMD, tag="xt", name="xt")
                    eng = nc.gpsimd if e % 2 == 0 else nc.scalar
                    eng.dma_start(out=t[:], in_=xT[e * 128:(e + 1) * 128, :])
                    xt.append(t)

                for bi, (kind, sub) in enumerate(blocks):
                    wb = wpool.tile([128, 2048], MMD, tag="w", name="w")
                    if bi == 0:
                        # fine-grained for fast PE start
                        for g4 in range(4):
                            nc.sync.dma_start(
                                out=wb[:, g4 * 512:(g4 + 1) * 512],
                                in_=Wil[:, bi * 2048 + g4 * 512:bi * 2048 + (g4 + 1) * 512])
                    else:
                        nc.sync.dma_start(out=wb[:], in_=Wil[:, bi * 2048:(bi + 1) * 2048])
                    pst = [m1ps.tile([128, 512], F32, tag="m1", name="m1")
                           for _ in range(NPC)]
                    for e in range(16):
                        for p in range(NPC):
                            nc.tensor.matmul(
                                pst[p][:], wb[:, e * 128:(e + 1) * 128],
                                xt[e][:, p * 512:(p + 1) * 512],
                                start=(e == 0), stop=(e == 15))
                    if bi == 0:
                        load_consts()
                    if bi == len(blocks) - 1:
                        for h in range(HPC):
                            nc.gpsimd.dma_start(out=wo_t[h][:],
                                                in_=Wo[h * 128:(h + 1) * 128, :])
                    for p in range(NPC):
                        ps = pst[p]
                        sl = slice(p * 512, (p + 1) * 512)
                        if kind == "v":
                            nc.scalar.copy(vT[:, sl], ps[:])
                        else:
                            if kind == "q":
                                h, a = sub
                                dst = qT[h][a]
                            else:
                                a = sub
                                dst = kT[a]
                            tmp = rtmp.tile([128, 512], F32, tag="swap", name="swap")
                            nc.vector.tensor_copy(tmp[0:64, :], ps[64:128, :])
                            nc.vector.tensor_copy(tmp[64:128, :], ps[0:64, :])
                            nc.vector.tensor_tensor(
                                dst[:, sl], ps[:], cos_t[a][:, sl], op=ALU.mult)
                            nc.gpsimd.tensor_tensor(
                                tmp[:], tmp[:], sin_t[a][:, sl], op=ALU.mult)
                            nc.vector.tensor_tensor(
                                dst[:, sl], dst[:, sl], tmp[:], op=ALU.add)

                # v: [d, pos] -> v_t[j]: [128 pos, 128 d] via DMA XBAR
                for j in range(NQT):
                    nc.sync.dma_start(out=v_t[j][:],
                                      in_=vT[:, j * 128:(j + 1) * 128],
                                      transpose=True)

            # ------------- phase B: attention per head -------------
            with ExitStack() as bctx:
                epool = bctx.enter_context(tc.tile_pool(name="expp", bufs=3))
                dtpool = bctx.enter_context(tc.tile_pool(name="difft", bufs=3))
                pvtr = bctx.enter_context(tc.tile_pool(name="pvtr", bufs=2, space="PSUM"))
                ssbp = bctx.enter_context(tc.tile_pool(name="ssbp", bufs=1, space="PSUM"))
                sm2 = bctx.enter_context(tc.tile_pool(name="sm2", bufs=2))
                ectx = ExitStack()
                eps = ectx.enter_context(tc.tile_pool(name="eps", bufs=5, space="PSUM"))

                diffTs = {}

                def emit_scores(h, pv_head=None):
                    # diffT[:, i, j, :] = scores^T block (k-block j, q-block i)
                    diffT = dtpool.tile([128, NQT, NQT, 128], BF16,
                                        tag="difft", name=f"difft{h}")
                    diffTs[h] = diffT

                    for i in range(NQT):
                        Ke = (i + 1) * 128
                        nch = 1 if Ke <= 512 else 2
                        # energy psum: chunk tiles (a: k<512, b: k>=512)
                        e0c = [eps.tile([128, 512], F32, tag="e", name="e0a")]
                        e1c = [eps.tile([128, 512], F32, tag="e", name="e1a")]
                        if nch == 2:
                            e0c.append(eps.tile([128, 512], F32, tag="e", name="e0b"))
                            e1c.append(eps.tile([128, 512], F32, tag="e", name="e1b"))
                        for a in range(2):
                            ec = e0c if a == 0 else e1c
                            for kc in range(nch):
                                w = min(Ke, (kc + 1) * 512) - kc * 512
                                ksl = slice(kc * 512, kc * 512 + w)
                                nc.tensor.matmul(
                                    ec[kc][:, 0:w],
                                    qT[h][a][:, i * 128:(i + 1) * 128],
                                    kT[a][:, ksl], start=True, stop=True)
                        # causal mask on the diagonal 128-block
                        dc, doff = (i * 128) // 512, (i * 128) % 512
                        dw = slice(doff, doff + 128)
                        nc.vector.tensor_tensor(e0c[dc][:, dw], e0c[dc][:, dw],
                                                mask_t[:], op=ALU.add)
                        nc.vector.tensor_tensor(e1c[dc][:, dw], e1c[dc][:, dw],
                                                mask_t[:], op=ALU.add)

                        exp0 = epool.tile([128, S], BF16, tag="exp0", name="exp0")
                        exp1 = epool.tile([128, S], BF16, tag="exp1", name="exp1")
                        s01p = [sm2.tile([128, 2], F32, tag="s01p", name="s01p", bufs=4)
                                for _ in range(nch)]
                        for kc in range(nch):
                            w = min(Ke, (kc + 1) * 512) - kc * 512
                            osl = slice(kc * 512, kc * 512 + w)
                            nc.scalar.activation(exp0[:, osl], e0c[kc][:, 0:w], ACT.Exp,
                                                 scale=ISCALE,
                                                 accum_out=s01p[kc][:, 0:1])
                            nc.scalar.activation(exp1[:, osl], e1c[kc][:, 0:w], ACT.Exp,
                                                 scale=ISCALE,
                                                 accum_out=s01p[kc][:, 1:2])
                        if nch == 2:
                            s01 = sm2.tile([128, 2], F32, tag="s01", name="s01")
                            nc.vector.tensor_tensor(s01[:], s01p[0][:], s01p[1][:],
                                                    op=ALU.add)
                        else:
                            s01 = s01p[0]
                        r01 = sm2.tile([128, 2], F32, tag="r01", name="r01")
                        nc.vector.reciprocal(r01[:], s01[:])
                        r1p = sm2.tile([128, 1], F32, tag="r1p", name="r1p")
                        nc.vector.scalar_tensor_tensor(
                            r1p[:], s01[:, 0:1], lam_t[:, h:h + 1], r01[:, 1:2],
                            op0=ALU.mult, op1=ALU.mult)
                        # t = (exp1 * r1p) + exp0 ; diag-masked; exp2 = Exp(r0*t)
                        t = epool.tile([128, S], BF16, tag="t", name="t")
                        nc.vector.scalar_tensor_tensor(
                            t[:, :Ke], exp1[:, :Ke], r1p[:], exp0[:, :Ke],
                            op0=ALU.mult, op1=ALU.add)
                        nc.vector.tensor_tensor(t[:, i * 128:(i + 1) * 128],
                                                t[:, i * 128:(i + 1) * 128],
                                                mask_b[:], op=ALU.add)
                        exp2 = epool.tile([128, S], BF16, tag="exp2", name="exp2")
                        for kc in range(nch):
                            w = min(Ke, (kc + 1) * 512) - kc * 512
                            osl = slice(kc * 512, kc * 512 + w)
                            nc.scalar.activation(exp2[:, osl], t[:, osl], ACT.Exp,
                                                 scale=r01[:, 0:1])
                        # inject prev head's PV here: pure PE filler (its
                        # diffT was fully written during the previous S())
                        if pv_head is not None and i == 6:
                            emit_pv_chunk(pv_head, 0)
                        if pv_head is not None and i == 7:
                            emit_pv_chunk(pv_head, 1)
                        # transpose scores into diffT via the DMA XBAR
                        # (out[p, b, q] = in[q, b*128+p]); frees PE + vector
                        for kc in range(nch):
                            w = min(Ke, (kc + 1) * 512) - kc * 512
                            nc.sync.dma_start(
                                out=diffT[:, i, 4 * kc:4 * kc + w // 128, :],
                                in_=exp2[:, kc * 512:kc * 512 + w],
                                transpose=True)

                def emit_pv_chunk(h, c):
                    # PV + RMS normalization for one 512-q chunk
                    # att_final = att_raw * sqrt(128/ss); softmax2 norm cancels
                    if True:
                        diffT = diffTs[h]
                        nk = 4 * c + 4
                        attps = pvtr.tile([128, 512], F32, tag="pvtr", name="att")
                        for j in range(nk):
                            imin = max(4 * c, j)
                            off = (imin - 4 * c) * 128
                            nc.tensor.matmul(
                                attps[:, off:512], v_t[j][:],
                                diffT[:, imin:4 * c + 4, j, :],
                                start=(j == 0), stop=(j == nk - 1))
                        # RMS over head_dim (partition axis): ones^T @ att^2 on
                        # the PE reduces over partitions AND broadcasts the sum
                        # to all 128 output partitions in one matmul.
                        # cfac = Dsqrt(ssq/512) = 0.5/sqrt(ssq/512) = rsqrt(ssq/128)
                        attsb = sm2.tile([128, 512], F32, tag="attsb", name="attsb")
                        nc.scalar.copy(attsb[:], attps[:])
                        att2 = sm2.tile([128, 512], BF16, tag="att2", name="att2")
                        nc.vector.tensor_tensor(att2[:], attsb[:], attsb[:],
                                                op=ALU.mult)
                        ssb = ssbp.tile([128, 512], F32, tag="ssb", name="ssb")
                        nc.tensor.matmul(ssb[:], ones_t[:], att2[:],
                                         start=True, stop=True)
                        rec = sm2.tile([128, 512], F32, tag="rec", name="rec")
                        nc.vector.reciprocal(rec[:], ssb[:])
                        cfac = sm2.tile([128, 512], F32, tag="cfac", name="cfac")
                        nc.scalar.activation(cfac[:], rec[:], ACT.Sqrt,
                                             scale=128.0)
                        nc.vector.tensor_tensor(
                            attf[h][c][:], attsb[:], cfac[:], op=ALU.mult)

                emit_scores(0)
                for h in range(1, HPC):
                    emit_scores(h, pv_head=h - 1)

                # tail: close the energy psum pool, then interleave the
                # last head's PV chunks with the output projection
                ectx.close()
                wops = bctx.enter_context(tc.tile_pool(name="wops", bufs=4, space="PSUM"))
                opool = bctx.enter_context(tc.tile_pool(name="osb", bufs=4))

                def emit_out(p):
                    c, po = p // 4, (p % 4) * 128
                    ops = [wops.tile([128, 512], F32, tag="o", name="o")
                           for _ in range(E // 512)]
                    for h in range(HPC):
                        for n in range(E // 512):
                            nc.tensor.matmul(
                                ops[n][:], attf[h][c][:, po:po + 128],
                                wo_t[h][:, n * 512:(n + 1) * 512],
                                start=(h == 0), stop=(h == HPC - 1))
                    for n in range(E // 512):
                        osb = opool.tile([128, 512], BF16, tag="osb", name="osb")
                        nc.vector.tensor_copy(osb[:], ops[n][:])
                        nc.sync.dma_start(
                            out=out_ext[p * 128:(p + 1) * 128, n * 512:(n + 1) * 512],
                            in_=osb[:])

                emit_pv_chunk(HPC - 1, 0)
                for p in range(4):
                    emit_out(p)
                emit_pv_chunk(HPC - 1, 1)
                for p in range(4, NQT):
                    emit_out(p)

    nc.finalize()
    return nc


def _host_prep(x, Wq, Wk, Wv, Wo, lq1, lq2, lk1, lk2, rms_w):
    lam = (np.exp((lq1 * lk1).sum(-1)) - np.exp((lq2 * lk2).sum(-1))
           + LAM_INIT).astype(np.float32)  # (H,)
    j = np.arange(D, dtype=np.float64)
    theta = 1.0 / (10000.0 ** (2.0 * j / (2 * D)))
    pos = np.arange(S, dtype=np.float64)
    ang = pos[None, :] * theta[:, None]  # (128, S)
    cosd = np.cos(ang).astype(np.float32)
    sin = np.sin(ang)
    cosd2 = np.concatenate([np.concatenate([cosd[a * 64:(a + 1) * 64]] * 2, 0)
                            for a in range(2)], 0)
    sind2 = np.concatenate(
        [np.concatenate([-sin[a * 64:(a + 1) * 64], sin[a * 64:(a + 1) * 64]], 0)
         for a in range(2)], 0).astype(np.float32)

    perm256 = np.concatenate([np.arange(0, 128, 2), np.arange(1, 128, 2),
                              np.arange(128, 256, 2), np.arange(129, 256, 2)])
    Wqp = Wq.reshape(E, H, 2 * D)[:, :, perm256].reshape(E, H * 2 * D)
    Wkp = Wk.reshape(E, KVH, 2 * D)[:, :, perm256].reshape(E, KVH * 2 * D)
    WoS = (Wo.reshape(H, D, E) * (rms_w[None, :, None] * (1.0 - LAM_INIT))
           ).reshape(E, E).astype(np.float32)

    maskn = np.where(np.arange(128)[None, :] > np.arange(128)[:, None],
                     np.float32(NEG), np.float32(0.0)).astype(np.float32)

    def interleave(Wcols):
        # [E, nb*128] -> [128, nb*2048]; block b, chunk e at cols b*2048+e*128
        nb = Wcols.shape[1] // 128
        return np.ascontiguousarray(
            Wcols.reshape(16, 128, nb, 128).transpose(1, 2, 0, 3)
            .reshape(128, nb * 2048))

    import ml_dtypes
    bf = ml_dtypes.bfloat16
    in_maps = []
    for core in range(NCORES):
        b, g = divmod(core, KVH)
        heads = slice(HPC * g * 2 * D, HPC * (g + 1) * 2 * D)
        lam_g = lam[HPC * g:HPC * (g + 1)]
        # block order must match kernel: k0,k1,v,q0..q7
        Wcat = np.concatenate([
            Wkp[:, g * 2 * D:(g + 1) * 2 * D],
            Wv[:, g * D:(g + 1) * D],
            Wqp[:, heads]], axis=1)
        in_maps.append({
            "xT": np.ascontiguousarray(x[b].T).astype(bf),
            "Wil": interleave(Wcat).astype(bf),
            "Wo": np.ascontiguousarray(WoS[HPC * D * g:HPC * D * (g + 1), :]).astype(bf),
            "cosd": cosd2.astype(bf),
            "sind": sind2.astype(bf),
            "lamn": np.tile(-lam_g[None, :], (D, 1)).astype(np.float32),
            "maskn": maskn,
        })
    return in_maps


def kernel(x, Wq, Wk, Wv, Wo, lq1, lq2, lk1, lk2, rms_w, _trace=False):
    from concourse import bass_utils

    in_maps = _host_prep(np.asarray(x, np.float32), np.asarray(Wq, np.float32),
                         np.asarray(Wk, np.float32), np.asarray(Wv, np.float32),
                         np.asarray(Wo, np.float32), np.asarray(lq1, np.float32),
                         np.asarray(lq2, np.float32), np.asarray(lk1, np.float32),
                         np.asarray(lk2, np.float32), np.asarray(rms_w, np.float32))
    if "nc" not in _cache:
        _cache["nc"] = _build()
    nc = _cache["nc"]
    res = bass_utils.run_bass_kernel_spmd(
        nc, in_maps, core_ids=list(range(NCORES)), trace=_trace)
    _cache["last_result"] = res
    parts = np.stack([np.asarray(res.results[c]["out"], dtype=np.float32)
                      for c in range(NCORES)], 0)
    out = parts.reshape(B, KVH, S, E).sum(1)
    return out.astype(np.float32)


# revision 25
# speedup vs baseline: 1.0400x; 1.0400x over previous
# Differential GQA attention layer (B=2, S=1024, E=2048, H=16, KVH=4, D=128)
# distributed over 8 TRN2 NeuronCores: shard = (batch b, kv-group g) so each
# core owns 1 batch x 4 query heads (1 kv head). All attention is core-local;
# the Wo row-sharded output projection partials are summed on the host.
#
# Self-contained: hardcodes shapes/sharding; builds+compiles a Bass/Tile
# kernel on first call and runs it via run_bass_kernel_spmd on cores 0-7.
import numpy as np

B, S, E, H, KVH = 2, 1024, 2048, 16, 4
D = 128
NEG = -1e30
LAM_INIT = 0.2  # 0.8 - 0.6*exp(-0.3*layer_idx), layer_idx=0
NCORES = 8
HPC = H // KVH  # heads per core = 4

MM_DT = "bf16"  # PE path dtype: bf16 LDWEIGHTS+MM is ~1.9x faster than f32/f32r
NBLK = 2 + 1 + 2 * HPC  # weight column blocks: k0,k1,v,q0..q7

_cache = {}


def _build(dbg=False):
    import concourse.mybir as mybir
    import concourse.tile as tile
    from concourse import bacc
    from concourse.masks import make_identity
    from contextlib import ExitStack

    F32 = mybir.dt.float32
    BF16 = mybir.dt.bfloat16
    MMD = BF16 if MM_DT == "bf16" else (mybir.dt.float32r if MM_DT == "f32r" else F32)
    ALU = mybir.AluOpType
    ACT = mybir.ActivationFunctionType

    nc = bacc.Bacc(None, target_bir_lowering=False)

    xT = nc.declare_dram_parameter("xT", [E, S], MMD, isOutput=False)
    # all projection weights, e-interleaved per 128-col block:
    # Wil[p, blk*2048 + e*128 + c] = W[e*128+p, blk_cols+c]
    Wil = nc.declare_dram_parameter("Wil", [128, NBLK * 2048], MMD, isOutput=False)
    Wo = nc.declare_dram_parameter("Wo", [HPC * D, E], MMD, isOutput=False)
    cosd = nc.declare_dram_parameter("cosd", [2 * D, S], BF16, isOutput=False)
    sind = nc.declare_dram_parameter("sind", [2 * D, S], BF16, isOutput=False)
    lamn = nc.declare_dram_parameter("lamn", [D, HPC], F32, isOutput=False)
    maskn = nc.declare_dram_parameter("maskn", [D, D], F32, isOutput=False)
    out_ext = nc.declare_dram_parameter("out", [S, E], BF16, isOutput=True)

    ISCALE = 1.0 / float(np.sqrt(D))
    NQT = S // 128
    NPC = S // 512

    with tile.TileContext(nc) as tc:
        with ExitStack() as ctx:
            cpool = ctx.enter_context(tc.tile_pool(name="const", bufs=1))
            qkpool = ctx.enter_context(tc.tile_pool(name="qk", bufs=1))
            smalls = ctx.enter_context(tc.tile_pool(name="smalls", bufs=2))

            # constants (tiles now; DMAs deferred until after the first
            # m1 block so xT/W loads own the queues at kernel start)
            cos_t = [cpool.tile([128, S], F32, tag=f"cos{a}", name=f"cos{a}") for a in range(2)]
            sin_t = [cpool.tile([128, S], F32, tag=f"sin{a}", name=f"sin{a}") for a in range(2)]
            cos_b = [cpool.tile([128, S], BF16, tag=f"cosb{a}", name=f"cosb{a}") for a in range(2)]
            sin_b = [cpool.tile([128, S], BF16, tag=f"sinb{a}", name=f"sinb{a}") for a in range(2)]
            lam_t = cpool.tile([128, HPC], F32, tag="lam", name="lam")
            mask_t = cpool.tile([128, 128], F32, tag="mask", name="mask")

            def load_consts():
                for a in range(2):
                    nc.sync.dma_start(out=cos_b[a][:], in_=cosd[a * 128:(a + 1) * 128, :])
                    nc.sync.dma_start(out=sin_b[a][:], in_=sind[a * 128:(a + 1) * 128, :])
                nc.sync.dma_start(out=lam_t[:], in_=lamn[:])
                nc.sync.dma_start(out=mask_t[:], in_=maskn[:])
                for a in range(2):
                    nc.scalar.copy(cos_t[a][:], cos_b[a][:])
                    nc.scalar.copy(sin_t[a][:], sin_b[a][:])
                nc.scalar.copy(mask_b[:], mask_t[:])
            ident = cpool.tile([128, 128], F32, tag="ident", name="ident")
            make_identity(nc, ident[:])
            identb = cpool.tile([128, 128], BF16, tag="identb", name="identb")
            make_identity(nc, identb[:])
            ones_t = cpool.tile([128, 128], BF16, tag="ones", name="ones")
            nc.gpsimd.memset(ones_t[:], 1.0)
            mask_b = cpool.tile([128, 128], BF16, tag="maskb", name="maskb")

            # persistent activations
            qT = [[qkpool.tile([128, S], MMD, tag=f"qT{h}{a}", name=f"qT{h}{a}")
                   for a in range(2)] for h in range(HPC)]
            kT = [qkpool.tile([128, S], MMD, tag=f"kT{a}", name=f"kT{a}") for a in range(2)]
            vT = qkpool.tile([128, S], MMD, tag="vT", name="vT")
            v_t = [qkpool.tile([128, 128], BF16, tag=f"v{j}", name=f"v{j}")
                   for j in range(NQT)]
            attf = [[qkpool.tile([128, 512], MMD, tag=f"attf{h}{c}", name=f"attf{h}{c}")
                     for c in range(NPC)] for h in range(HPC)]
            wo_t = [qkpool.tile([128, E], MMD, tag=f"wo{h}", name=f"wo{h}")
                    for h in range(HPC)]

            # ------------- phase A: x @ W -> qT/kT/vT (+rope), v -------------
            with ExitStack() as actx:
                xpool = actx.enter_context(tc.tile_pool(name="xT", bufs=16))
                wpool = actx.enter_context(tc.tile_pool(name="w", bufs=3))
                m1ps = actx.enter_context(tc.tile_pool(name="m1ps", bufs=6, space="PSUM"))
                vtrps = actx.enter_context(tc.tile_pool(name="vtrps", bufs=2, space="PSUM"))
                rtmp = actx.enter_context(tc.tile_pool(name="rtmp", bufs=3))

                # k and v first so phase B can start while q streams
                blocks = []
                for a in range(2):
                    blocks.append(("k", a))
                blocks.append(("v", None))
                for h in range(HPC):
                    for a in range(2):
                        blocks.append(("q", (h, a)))

                # xT tiles on the gpsimd DMA queue (parallel with weights
                # on the sync queue)
                xt = []
                for e in range(16):
                    t = xpool.tile([128, S], MMD, tag="xt", name="xt")
                    eng = nc.gpsimd if e % 2 == 0 else nc.scalar
                    eng.dma_start(out=t[:], in_=xT[e * 128:(e + 1) * 128, :])
                    xt.append(t)

                for bi, (kind, sub) in enumerate(blocks):
                    wb = wpool.tile([128, 2048], MMD, tag="w", name="w")
                    if bi == 0:
                        # fine-grained for fast PE start
                        for g4 in range(4):
                            nc.sync.dma_start(
                                out=wb[:, g4 * 512:(g4 + 1) * 512],
                                in_=Wil[:, bi * 2048 + g4 * 512:bi * 2048 + (g4 + 1) * 512])
                    else:
                        nc.sync.dma_start(out=wb[:], in_=Wil[:, bi * 2048:(bi + 1) * 2048])
                    pst = [m1ps.tile([128, 512], F32, tag="m1", name="m1")
                           for _ in range(NPC)]
                    for e in range(16):
                        for p in range(NPC):
                            nc.tensor.matmul(
                                pst[p][:], wb[:, e * 128:(e + 1) * 128],
                                xt[e][:, p * 512:(p + 1) * 512],
                                start=(e == 0), stop=(e == 15))
                    if bi == 0:
                        load_consts()
                    if bi == len(blocks) - 1:
                        for h in range(HPC):
                            nc.sync.dma_start(out=wo_t[h][:],
                                              in_=Wo[h * 128:(h + 1) * 128, :])
                    for p in range(NPC):
                        ps = pst[p]
                        sl = slice(p * 512, (p + 1) * 512)
                        if kind == "v":
                            nc.scalar.copy(vT[:, sl], ps[:])
                        else:
                            if kind == "q":
                                h, a = sub
                                dst = qT[h][a]
                            else:
                                a = sub
                                dst = kT[a]
                            tmp = rtmp.tile([128, 512], F32, tag="swap", name="swap")
                            nc.vector.tensor_copy(tmp[0:64, :], ps[64:128, :])
                            nc.vector.tensor_copy(tmp[64:128, :], ps[0:64, :])
                            nc.vector.tensor_tensor(
                                dst[:, sl], ps[:], cos_t[a][:, sl], op=ALU.mult)
                            nc.gpsimd.tensor_tensor(
                                tmp[:], tmp[:], sin_t[a][:, sl], op=ALU.mult)
                            nc.vector.tensor_tensor(
                                dst[:, sl], dst[:, sl], tmp[:], op=ALU.add)

                # v: [d, pos] -> v_t[j]: [128 pos, 128 d] via DMA XBAR
                for j in range(NQT):
                    nc.sync.dma_start(out=v_t[j][:],
                                      in_=vT[:, j * 128:(j + 1) * 128],
                                      transpose=True)

            # ------------- phase B: attention per head -------------
            with ExitStack() as bctx:
                epool = bctx.enter_context(tc.tile_pool(name="expp", bufs=3))
                dtpool = bctx.enter_context(tc.tile_pool(name="difft", bufs=3))
                pvtr = bctx.enter_context(tc.tile_pool(name="pvtr", bufs=2, space="PSUM"))
                ssbp = bctx.enter_context(tc.tile_pool(name="ssbp", bufs=1, space="PSUM"))
                sm2 = bctx.enter_context(tc.tile_pool(name="sm2", bufs=2))
                ectx = ExitStack()
                eps = ectx.enter_context(tc.tile_pool(name="eps", bufs=5, space="PSUM"))

                diffTs = {}

                def emit_scores(h, pv_head=None):
                    # diffT[:, i, j, :] = scores^T block (k-block j, q-block i)
                    diffT = dtpool.tile([128, NQT, NQT, 128], BF16,
                                        tag="difft", name=f"difft{h}")
                    diffTs[h] = diffT

                    for i in range(NQT):
                        Ke = (i + 1) * 128
                        nch = 1 if Ke <= 512 else 2
                        # energy psum: chunk tiles (a: k<512, b: k>=512)
                        e0c = [eps.tile([128, 512], F32, tag="e", name="e0a")]
                        e1c = [eps.tile([128, 512], F32, tag="e", name="e1a")]
                        if nch == 2:
                            e0c.append(eps.tile([128, 512], F32, tag="e", name="e0b"))
                            e1c.append(eps.tile([128, 512], F32, tag="e", name="e1b"))
                        for a in range(2):
                            ec = e0c if a == 0 else e1c
                            for kc in range(nch):
                                w = min(Ke, (kc + 1) * 512) - kc * 512
                                ksl = slice(kc * 512, kc * 512 + w)
                                nc.tensor.matmul(
                                    ec[kc][:, 0:w],
                                    qT[h][a][:, i * 128:(i + 1) * 128],
                                    kT[a][:, ksl], start=True, stop=True)
                        # causal mask on the diagonal 128-block
                        dc, doff = (i * 128) // 512, (i * 128) % 512
                        dw = slice(doff, doff + 128)
                        nc.vector.tensor_tensor(e0c[dc][:, dw], e0c[dc][:, dw],
                                                mask_t[:], op=ALU.add)
                        nc.vector.tensor_tensor(e1c[dc][:, dw], e1c[dc][:, dw],
                                                mask_t[:], op=ALU.add)

                        exp0 = epool.tile([128, S], BF16, tag="exp0", name="exp0")
                        exp1 = epool.tile([128, S], BF16, tag="exp1", name="exp1")
                        s01p = [sm2.tile([128, 2], F32, tag="s01p", name="s01p", bufs=4)
                                for _ in range(nch)]
                        for kc in range(nch):
                            w = min(Ke, (kc + 1) * 512) - kc * 512
                            osl = slice(kc * 512, kc * 512 + w)
                            nc.scalar.activation(exp0[:, osl], e0c[kc][:, 0:w], ACT.Exp,
                                                 scale=ISCALE,
                                                 accum_out=s01p[kc][:, 0:1])
                            nc.scalar.activation(exp1[:, osl], e1c[kc][:, 0:w], ACT.Exp,
                                                 scale=ISCALE,
                                                 accum_out=s01p[kc][:, 1:2])
                        if nch == 2:
                            s01 = sm2.tile([128, 2], F32, tag="s01", name="s01")
                            nc.vector.tensor_tensor(s01[:], s01p[0][:], s01p[1][:],
                                                    op=ALU.add)
                        else:
                            s01 = s01p[0]
                        r01 = sm2.tile([128, 2], F32, tag="r01", name="r01")
                        nc.vector.reciprocal(r01[:], s01[:])
                        r1p = sm2.tile([128, 1], F32, tag="r1p", name="r1p")
                        nc.vector.scalar_tensor_tensor(
                            r1p[:], s01[:, 0:1], lam_t[:, h:h + 1], r01[:, 1:2],
                            op0=ALU.mult, op1=ALU.mult)
                        # t = (exp1 * r1p) + exp0 ; diag-masked; exp2 = Exp(r0*t)
                        t = epool.tile([128, S], BF16, tag="t", name="t")
                        nc.vector.scalar_tensor_tensor(
                            t[:, :Ke], exp1[:, :Ke], r1p[:], exp0[:, :Ke],
                            op0=ALU.mult, op1=ALU.add)
                        nc.vector.tensor_tensor(t[:, i * 128:(i + 1) * 128],
                                                t[:, i * 128:(i + 1) * 128],
                                                mask_b[:], op=ALU.add)
                        exp2 = epool.tile([128, S], BF16, tag="exp2", name="exp2")
                        for kc in range(nch):
                            w = min(Ke, (kc + 1) * 512) - kc * 512
                            osl = slice(kc * 512, kc * 512 + w)
                            nc.scalar.activation(exp2[:, osl], t[:, osl], ACT.Exp,
                                                 scale=r01[:, 0:1])
                        # inject prev head's PV here: pure PE filler (its
                        # diffT was fully written during the previous S())
                        if pv_head is not None and i == 6:
                            emit_pv_chunk(pv_head, 0)
                        if pv_head is not None and i == 7:
                            emit_pv_chunk(pv_head, 1)
                        # transpose scores into diffT via the DMA XBAR
                        # (out[p, b, q] = in[q, b*128+p]); frees PE + vector
                        for kc in range(nch):
                            w = min(Ke, (kc + 1) * 512) - kc * 512
                            nc.sync.dma_start(
                                out=diffT[:, i, 4 * kc:4 * kc + w // 128, :],
                                in_=exp2[:, kc * 512:kc * 512 + w],
                                transpose=True)

                def emit_pv_chunk(h, c):
                    # PV + RMS normalization for one 512-q chunk
                    # att_final = att_raw * sqrt(128/ss); softmax2 norm cancels
                    if True:
                        diffT = diffTs[h]
                        nk = 4 * c + 4
                        attps = pvtr.tile([128, 512], F32, tag="pvtr", name="att")
                        for j in range(nk):
                            imin = max(4 * c, j)
                            off = (imin - 4 * c) * 128
                            nc.tensor.matmul(
                                attps[:, off:512], v_t[j][:],
                                diffT[:, imin:4 * c + 4, j, :],
                                start=(j == 0), stop=(j == nk - 1))
                        # RMS over head_dim (partition axis): ones^T @ att^2 on
                        # the PE reduces over partitions AND broadcasts the sum
                        # to all 128 output partitions in one matmul.
                        # cfac = Dsqrt(ssq/512) = 0.5/sqrt(ssq/512) = rsqrt(ssq/128)
                        attsb = sm2.tile([128, 512], F32, tag="attsb", name="attsb")
                        nc.scalar.copy(attsb[:], attps[:])
                        att2 = sm2.tile([128, 512], BF16, tag="att2", name="att2")
                        nc.vector.tensor_tensor(att2[:], attsb[:], attsb[:],
                                                op=ALU.mult)
                        ssb = ssbp.tile([128, 512], F32, tag="ssb", name="ssb")
                        nc.tensor.matmul(ssb[:], ones_t[:], att2[:],
                                         start=True, stop=True)
                        rec = sm2.tile([128, 512], F32, tag="rec", name="rec")
                        nc.vector.reciprocal(rec[:], ssb[:])
                        cfac = sm2.tile([128, 512], F32, tag="cfac", name="cfac")
                        nc.scalar.activation(cfac[:], rec[:], ACT.Sqrt,
                                             scale=128.0)
                        nc.vector.tensor_tensor(
                            attf[h][c][:], attsb[:], cfac[:], op=ALU.mult)

                emit_scores(0)
                for h in range(1, HPC):
                    emit_scores(h, pv_head=h - 1)

                # tail: close the energy psum pool, then interleave the
                # last head's PV chunks with the output projection
                ectx.close()
                wops = bctx.enter_context(tc.tile_pool(name="wops", bufs=4, space="PSUM"))
                opool = bctx.enter_context(tc.tile_pool(name="osb", bufs=4))

                def emit_out(p):
                    c, po = p // 4, (p % 4) * 128
                    ops = [wops.tile([128, 512], F32, tag="o", name="o")
                           for _ in range(E // 512)]
                    for h in range(HPC):
                        for n in range(E // 512):
                            nc.tensor.matmul(
                                ops[n][:], attf[h][c][:, po:po + 128],
                                wo_t[h][:, n * 512:(n + 1) * 512],
                                start=(h == 0), stop=(h == HPC - 1))
                    for n in range(E // 512):
                        osb = opool.tile([128, 512], BF16, tag="osb", name="osb")
                        nc.scalar.copy(osb[:], ops[n][:])
                        nc.sync.dma_start(
                            out=out_ext[p * 128:(p + 1) * 128, n * 512:(n + 1) * 512],
                            in_=osb[:])

                emit_pv_chunk(HPC - 1, 0)
                for p in range(4):
                    emit_out(p)
                emit_pv_chunk(HPC - 1, 1)
                for p in range(4, NQT):
                    emit_out(p)

    nc.finalize()
    return nc


def _host_prep(x, Wq, Wk, Wv, Wo, lq1, lq2, lk1, lk2, rms_w):
    lam = (np.exp((lq1 * lk1).sum(-1)) - np.exp((lq2 * lk2).sum(-1))
           + LAM_INIT).astype(np.float32)  # (H,)
    j = np.arange(D, dtype=np.float64)
    theta = 1.0 / (10000.0 ** (2.0 * j / (2 * D)))
    pos = np.arange(S, dtype=np.float64)
    ang = pos[None, :] * theta[:, None]  # (128, S)
    cosd = np.cos(ang).astype(np.float32)
    sin = np.sin(ang)
    cosd2 = np.concatenate([np.concatenate([cosd[a * 64:(a + 1) * 64]] * 2, 0)
                            for a in range(2)], 0)
    sind2 = np.concatenate(
        [np.concatenate([-sin[a * 64:(a + 1) * 64], sin[a * 64:(a + 1) * 64]], 0)
         for a in range(2)], 0).astype(np.float32)

    perm256 = np.concatenate([np.arange(0, 128, 2), np.arange(1, 128, 2),
                              np.arange(128, 256, 2), np.arange(129, 256, 2)])
    Wqp = Wq.reshape(E, H, 2 * D)[:, :, perm256].reshape(E, H * 2 * D)
    Wkp = Wk.reshape(E, KVH, 2 * D)[:, :, perm256].reshape(E, KVH * 2 * D)
    WoS = (Wo.reshape(H, D, E) * (rms_w[None, :, None] * (1.0 - LAM_INIT))
           ).reshape(E, E).astype(np.float32)

    maskn = np.where(np.arange(128)[None, :] > np.arange(128)[:, None],
                     np.float32(NEG), np.float32(0.0)).astype(np.float32)

    def interleave(Wcols):
        # [E, nb*128] -> [128, nb*2048]; block b, chunk e at cols b*2048+e*128
        nb = Wcols.shape[1] // 128
        return np.ascontiguousarray(
            Wcols.reshape(16, 128, nb, 128).transpose(1, 2, 0, 3)
            .reshape(128, nb * 2048))

    import ml_dtypes
    bf = ml_dtypes.bfloat16
    in_maps = []
    for core in range(NCORES):
        b, g = divmod(core, KVH)
        heads = slice(HPC * g * 2 * D, HPC * (g + 1) * 2 * D)
        lam_g = lam[HPC * g:HPC * (g + 1)]
        # block order must match kernel: k0,k1,v,q0..q7
        Wcat = np.concatenate([
            Wkp[:, g * 2 * D:(g + 1) * 2 * D],
            Wv[:, g * D:(g + 1) * D],
            Wqp[:, heads]], axis=1)
        in_maps.append({
            "xT": np.ascontiguousarray(x[b].T).astype(bf),
            "Wil": interleave(Wcat).astype(bf),
            "Wo": np.ascontiguousarray(WoS[HPC * D * g:HPC * D * (g + 1), :]).astype(bf),
            "cosd": cosd2.astype(bf),
            "sind": sind2.astype(bf),
            "lamn": np.tile(-lam_g[None, :], (D, 1)).astype(np.float32),
            "maskn": maskn,
        })
    return in_maps


def kernel(x, Wq, Wk, Wv, Wo, lq1, lq2, lk1, lk2, rms_w, _trace=False):
    from concourse import bass_utils

    in_maps = _host_prep(np.asarray(x, np.float32), np.asarray(Wq, np.float32),
                         np.asarray(Wk, np.float32), np.asarray(Wv, np.float32),
                         np.asarray(Wo, np.float32), np.asarray(lq1, np.float32),
                         np.asarray(lq2, np.float32), np.asarray(lk1, np.float32),
                         np.asarray(lk2, np.float32), np.asarray(rms_w, np.float32))
    if "nc" not in _cache:
        _cache["nc"] = _build()
    nc = _cache["nc"]
    res = bass_utils.run_bass_kernel_spmd(
        nc, in_maps, core_ids=list(range(NCORES)), trace=_trace)
    _cache["last_result"] = res
    parts = np.stack([np.asarray(res.results[c]["out"], dtype=np.float32)
                      for c in range(NCORES)], 0)
    out = parts.reshape(B, KVH, S, E).sum(1)
    return out.astype(np.float32)
